# revision 1
# baseline (speedup 1.0000x reference)
"""Segment-mean pooling (segment_sum / counts) + Linear, on 8 TRN2 NeuronCores.

Strategy: segment-ownership sharding.  The host sorts rows by dst_idx and
routes each row to the core that owns its segment range (core i owns
segments [512*i, 512*(i+1))), so no collectives are needed; the host
concatenates the 8 output shards.

Per core, the segment sums are computed in [segment, hidden] layout
(segments on PSUM partitions) in two passes:

  Pass 1 (banded): the host packs the first C=16 rows of every segment
  into a dense band of 16-row slots (~98% full).  A 128-row chunk then
  covers exactly 8 consecutive segments, and its segment-sum is ONE
  TensorE matmul: stationary = a constant block-ones [128, 32] matrix,
  moving = the x rows [128, 256].  No per-row index handling at all.

  Pass 2 (one-hot tail): rows beyond slot 16 (~4% of rows) go through
  windowed one-hot matmuls: VectorE builds is_equal one-hots against an
  iota row (precomputed during pass 1), and each chunk's matmuls write
  narrow 32-aligned windows of the accumulators.  The window schedule is
  shared across cores (min/max over cores) so the SPMD graph is
  identical on every core.

Both band and overflow arrays are shipped pre-swizzled as [128, k, 256]
so every DMA is a fully linear copy.  PSUM accumulators are zero-opened
by rank-1 matmuls, so all data matmuls are pure accumulates in any
order.  Every PSUM tensor is padded to a full private 2 KiB bank, and
VectorE only reads a bank once all TensorE writes to it are complete
(PE-write + DVE-read on one bank is a fatal HW error).

Epilogue: scale rows by 1/(count+eps) (host bincount shipped as a
reciprocal table), PE-transpose pooled to [hidden, segment], apply the
Linear as out[s, j] = pooled_T[:, s].T @ W.T[h, j] with fused bias-add
(per-tile pipelined), and DMA the [512, 256] f32 shard.
"""

import os

import numpy as np

import concourse.bass as bass
import concourse.mybir as mybir
from concourse.bass_utils import run_bass_kernel_spmd

N_CORES = 8
S_TOTAL = 4096
S_PER = S_TOTAL // N_CORES  # 512 segments per core
H = 256
EPS = np.float32(1e-8)
PAD_IDX = 9999.0  # sentinel relative idx; never matches iota [0, wmax2)
C = 16  # band-A capacity (rows per segment); must divide 128
C2 = 8  # band-B capacity (rows 16..24 of a segment); must divide 128

GSZ = 8  # chunks per band DMA (1024 rows, 512 KB)
KB = S_PER * C // 128  # 64 band-A chunks
KB2 = S_PER * C2 // 128  # 32 band-B chunks
N_BAND_GROUPS = KB // GSZ  # 8
N_B2_GROUPS = KB2 // GSZ  # 4

_graph_cache: dict = {}

if os.environ.get("K_LDW"):
    try:
        import libneuronxla.libncc as _ncc

        _ncc.NEURON_CC_FLAGS = [
            f.replace("--enable-ldw-opt=false", "--enable-ldw-opt=true")
            for f in _ncc.NEURON_CC_FLAGS
        ]
        os.environ["AXON_NCC_FLAGS"] = os.environ.get("AXON_NCC_FLAGS", "").replace(
            "--enable-ldw-opt=false", "--enable-ldw-opt=true"
        )
    except Exception:
        pass


def _build(ov_chunks: int, ov_parts: tuple, wmax2: int) -> "bass.Bass":
    """ov_parts[oc] = tuple of 32-aligned window-part start segments."""
    f16 = mybir.dt.float16
    f32 = mybir.dt.float32
    ovk = max(ov_chunks, 1)

    nc = bass.Bass()

    xb_d = nc.declare_dram_parameter("xb", [128, KB, H], f16, isOutput=False)
    xb2_d = nc.declare_dram_parameter("xb2", [128, KB2, H], f16, isOutput=False)
    xov_d = nc.declare_dram_parameter("xov", [128, ovk, H], f16, isOutput=False)
    ovidx_d = nc.declare_dram_parameter("ovidx", [128, ovk], f32, isOutput=False)
    iota_d = nc.declare_dram_parameter("iota", [128, wmax2 + 256], f16, isOutput=False)
    ones_d = nc.declare_dram_parameter("ones32", [128, 6, 32], f16, isOutput=False)
    ident_d = nc.declare_dram_parameter("ident", [128, 128], f16, isOutput=False)
    wt_d = nc.declare_dram_parameter("wt", [H, H], f16, isOutput=False)
    invc_d = nc.declare_dram_parameter("invc", [128, 4], f32, isOutput=False)
    bb_d = nc.declare_dram_parameter("bb", [128, H], f32, isOutput=False)
    out_d = nc.declare_dram_parameter("out", [S_PER, H], f32, isOutput=True)

    from contextlib import ExitStack

    with ExitStack() as ctx:
        xbb = ctx.enter_context(nc.sbuf_tensor("xbb", [128, KB, H], f16))
        xbb2 = ctx.enter_context(nc.sbuf_tensor("xbb2", [128, KB2, H], f16))
        xov_sb = ctx.enter_context(nc.sbuf_tensor("xov_sb", [128, ovk, H], f16))
        oh2 = ctx.enter_context(nc.sbuf_tensor("oh2", [128, ovk, wmax2], f16))
        ovidx_sb = ctx.enter_context(nc.sbuf_tensor("ovidx_sb", [128, ovk], f32))
        iota_sb = ctx.enter_context(nc.sbuf_tensor("iota_sb", [128, wmax2 + 256], f16))
        ones_sb = ctx.enter_context(nc.sbuf_tensor("ones_sb", [128, 6, 32], f16))
        ident_sb = ctx.enter_context(nc.sbuf_tensor("ident_sb", [128, 128], f16))
        wt_sb = ctx.enter_context(nc.sbuf_tensor("wt_sb", [128, 2, H], f16))
        invc_sb = ctx.enter_context(nc.sbuf_tensor("invc_sb", [128, 4], f32))
        bb_sb = ctx.enter_context(nc.sbuf_tensor("bb_sb", [128, H], f32))
        pool_sb = ctx.enter_context(nc.sbuf_tensor("pool_sb", [128, 4, H], f16))
        sums2_sb = ctx.enter_context(nc.sbuf_tensor("sums2_sb", [128, 2, S_PER], f16))
        out_sb = ctx.enter_context(nc.sbuf_tensor("out_sb", [128, 4, H], f32))
        # every PSUM tensor padded to one full private 2 KiB bank
        ps_s = [
            ctx.enter_context(nc.psum_tensor(f"ps_s{t}", [128, 512], f32))
            for t in range(4)
        ]
        ps_t = [
            ctx.enter_context(nc.psum_tensor(f"ps_t{hb}", [128, 1024], f16))
            for hb in range(2)
        ]
        ps_x = ctx.enter_context(nc.psum_tensor("ps_x", [128, 512], f32))
        dma_sem = ctx.enter_context(nc.semaphore("dma_sem"))
        csem = {
            name: ctx.enter_context(nc.semaphore(f"csem_{name}"))
            for name in ("iota", "ovidx", "ones", "ident", "wt", "invc", "bb")
        }
        bsem = [
            ctx.enter_context(nc.semaphore(f"bsem{g}"))
            for g in range(N_BAND_GROUPS)
        ]
        b2sem = [
            ctx.enter_context(nc.semaphore(f"b2sem{g}"))
            for g in range(N_B2_GROUPS)
        ]
        xsem = ctx.enter_context(nc.semaphore("xsem"))
        b2last = ctx.enter_context(nc.semaphore("b2last"))
        cmp_sem = ctx.enter_context(nc.semaphore("cmp_sem"))
        mm_sem = ctx.enter_context(nc.semaphore("mm_sem"))
        cp_sem = ctx.enter_context(nc.semaphore("cp_sem"))
        tr_sem = ctx.enter_context(nc.semaphore("tr_sem"))
        cp2_sem = ctx.enter_context(nc.semaphore("cp2_sem"))
        mme_sem = ctx.enter_context(nc.semaphore("mme_sem"))
        oe_sem = ctx.enter_context(nc.semaphore("oe_sem"))
        block = ctx.enter_context(nc.Block())

        zlhs = iota_sb[0:1, 0:128]  # junk values; multiplied by zero rhs
        zrhs = iota_sb[0:1, wmax2 : wmax2 + 256]  # zeros

        @block.sync
        def _(sync):
            # late-needed consts on the sync ring
            sync.dma_start(out=ident_sb[:, :], in_=ident_d[:, :]).then_inc(
                csem["ident"], 16
            )
            sync.dma_start(
                out=wt_sb[:, :, :],
                in_=wt_d[:, :].rearrange("(t p) j -> p t j", p=128),
            ).then_inc(csem["wt"], 16)
            sync.dma_start(out=invc_sb[:, :], in_=invc_d[:, :]).then_inc(
                csem["invc"], 16
            )
            sync.dma_start(out=bb_sb[:, :], in_=bb_d[:, :]).then_inc(csem["bb"], 16)
            for st in range(4):
                sync.wait_ge(oe_sem, st + 1)
                sync.dma_start(
                    out=out_d[st * 128 : (st + 1) * 128, :], in_=out_sb[:, st, :]
                ).then_inc(dma_sem, 16)
            for name in ("ident", "wt", "invc", "bb"):
                sync.wait_ge(csem[name], 16)
            sync.wait_ge(dma_sem, 16 * 4)

        @block.scalar
        def _(scalar):
            # ALL input DMAs on one ring, in consumption order, one
            # semaphore per DMA: cumulative thresholds on a shared sem
            # can't tell WHICH transfer completed.
            scalar.dma_start(out=ones_sb[:, :, :], in_=ones_d[:, :, :]).then_inc(
                csem["ones"], 16
            )
            scalar.dma_start(out=iota_sb[:, :], in_=iota_d[:, :]).then_inc(
                csem["iota"], 16
            )
            scalar.dma_start(out=ovidx_sb[:, :], in_=ovidx_d[:, :]).then_inc(
                csem["ovidx"], 16
            )
            scalar.dma_start(out=xov_sb[:, :, :], in_=xov_d[:, :, :]).then_inc(
                xsem, 16
            )
            for g in range(N_BAND_GROUPS):
                scalar.dma_start(
                    out=xbb[:, GSZ * g : GSZ * (g + 1), :],
                    in_=xb_d[:, GSZ * g : GSZ * (g + 1), :],
                ).then_inc(bsem[g], 16)
            for g in range(N_B2_GROUPS - 1):
                scalar.dma_start(
                    out=xbb2[:, GSZ * g : GSZ * (g + 1), :],
                    in_=xb2_d[:, GSZ * g : GSZ * (g + 1), :],
                ).then_inc(b2sem[g], 16)
            gl = N_B2_GROUPS - 1
            scalar.dma_start(
                out=xbb2[:, GSZ * gl : GSZ * gl + 4, :],
                in_=xb2_d[:, GSZ * gl : GSZ * gl + 4, :],
            ).then_inc(b2sem[gl], 16)
            scalar.dma_start(
                out=xbb2[:, GSZ * gl + 4 : GSZ * (gl + 1), :],
                in_=xb2_d[:, GSZ * gl + 4 : GSZ * (gl + 1), :],
            ).then_inc(b2last, 16)
            for g in range(N_BAND_GROUPS):
                scalar.wait_ge(bsem[g], 16)
            for g in range(N_B2_GROUPS - 1):
                scalar.wait_ge(b2sem[g], 16)
            scalar.wait_ge(b2sem[N_B2_GROUPS - 1], 16)
            scalar.wait_ge(b2last, 16)
            scalar.wait_ge(xsem, 16)
            for name in ("ones", "iota", "ovidx"):
                scalar.wait_ge(csem[name], 16)

        @block.vector
        def _(vector):
            # pass-2 one-hots, precomputed while PE runs the band pass
            if ov_chunks:
                vector.wait_ge(csem["iota"], 16)
                vector.wait_ge(csem["ovidx"], 16)
                for oc in range(ov_chunks):
                    woc = 32 * len(ov_parts[oc])
                    vector.tensor_scalar(
                        out=oh2[:, oc, 0:woc],
                        in0=iota_sb[:, 0:woc],
                        scalar1=ovidx_sb[:, oc : oc + 1],
                        scalar2=None,
                        op0=mybir.AluOpType.is_equal,
                    ).then_inc(cmp_sem, 1)
            # epilogue
            vector.wait_ge(mm_sem, 1)  # all accumulation done
            for st in range(4):
                vector.tensor_copy(
                    out=pool_sb[:, st, :], in_=ps_s[st][:, 0:H]
                ).then_inc(cp_sem, 1)
            # ps_t banks are PE-owned until ALL transposes finish
            vector.wait_ge(tr_sem, 4)
            for st in range(4):
                vector.tensor_copy(
                    out=sums2_sb[:, 0, 128 * st : 128 * (st + 1)],
                    in_=ps_t[0][:, 128 * st : 128 * (st + 1)],
                )
                vector.tensor_copy(
                    out=sums2_sb[:, 1, 128 * st : 128 * (st + 1)],
                    in_=ps_t[1][:, 128 * st : 128 * (st + 1)],
                ).then_inc(cp2_sem, 1)
            vector.wait_ge(csem["invc"], 16)
            vector.wait_ge(csem["bb"], 16)
            for st in range(4):
                vector.wait_ge(mme_sem, st + 1)
                vector.scalar_tensor_tensor(
                    out=out_sb[:, st, :],
                    in0=ps_s[st][:, 0:H],
                    scalar=invc_sb[:, st : st + 1],
                    in1=bb_sb[:, :],
                    op0=mybir.AluOpType.mult,
                    op1=mybir.AluOpType.add,
                ).then_inc(oe_sem, 1)

        @block.tensor
        def _(tensor):
            tensor.wait_ge(csem["iota"], 16)
            tensor.wait_ge(csem["ones"], 16)
            # warm the PE clock (HAM) while the first data DMAs are in
            # flight: ~3.4us of sustained matmul activity moves the PE
            # from 1.2 GHz to 2.4 GHz for the whole band pass
            for _ in range(14):
                tensor.matmul(
                    ps_x[:, 0:256], ident_sb[:, :], iota_sb[:, 0:256],
                    start=True, stop=True, skip_group_check=True,
                )
            # zero-open all four accumulators
            for t in range(4):
                tensor.matmul(
                    ps_s[t][:, 0:H], zlhs, zrhs, start=True, stop=False,
                    skip_group_check=True,
                )
            # overflow one-hot pass first: runs while the bands stream in
            if ov_chunks:
                tensor.wait_ge(xsem, 16)
                for oc in range(ov_chunks):
                    tensor.wait_ge(cmp_sem, oc + 1)
                    for pi, seg0 in enumerate(ov_parts[oc]):
                        t, poff = seg0 // 128, seg0 % 128
                        tensor.matmul(
                            ps_s[t][poff : poff + 32, 0:H],
                            oh2[:, oc, 32 * pi : 32 * (pi + 1)],
                            xov_sb[:, oc, :],
                            start=False,
                            stop=False,
                            skip_group_check=True,
                            tile_position=(0, poff),
                        )
            # band A: chunk c covers segs [8c, 8c+8)
            for c in range(KB):
                if c % GSZ == 0:
                    tensor.wait_ge(bsem[c // GSZ], 16)
                    # full-width pulse so the HAM keeps the PE at 2.4 GHz
                    # (M=32 band matmuls alone do not register as busy)
                    tensor.matmul(
                        ps_x[:, 0:512], ident_sb[:, :],
                        xbb[:, GSZ * (c // GSZ) : GSZ * (c // GSZ) + 2, :],
                        start=True, stop=True, skip_group_check=True,
                    )
                v, j = divmod(c, 16)
                t, poff = j // 4, 32 * (j % 4)
                tensor.matmul(
                    ps_s[t][poff : poff + 32, 0:H],
                    ones_sb[:, v, :],
                    xbb[:, c, :],
                    start=False,
                    stop=False,
                    skip_group_check=True,
                    tile_position=(0, poff),
                )
            # band B: chunk c covers segs [16c, 16c+16)
            for c in range(KB2):
                g2 = c // GSZ
                if c % GSZ == 0:
                    tensor.wait_ge(b2sem[g2], 16)
                    tensor.matmul(
                        ps_x[:, 0:256], ident_sb[:, :], iota_sb[:, 0:256],
                        start=True, stop=True, skip_group_check=True,
                    )
                if g2 == N_B2_GROUPS - 1 and c % GSZ == 4:
                    tensor.wait_ge(b2last, 16)
                u, j = divmod(c, 16)
                t, poff = j // 4, 32 * (j % 4)
                tensor.matmul(
                    ps_s[t][poff : poff + 32, 0:H],
                    ones_sb[:, 4 + u, :],
                    xbb2[:, c, :],
                    start=False,
                    stop=False,
                    skip_group_check=True,
                    tile_position=(0, poff),
                )
            # close the accumulators
            for t in range(4):
                tensor.matmul(
                    ps_s[t][:, 0:H], zlhs, zrhs, start=False, stop=True,
                    skip_group_check=True,
                )
            # fence: matmul ends are FIFO; a matmul's then_inc can fire
            # before its PSUM writes drain, so hand banks to DVE only
            # after a trailing fence matmul completes
            tensor.matmul(
                ps_x[:, 0:H], zlhs, zrhs, start=True, stop=True,
                skip_group_check=True,
            ).then_inc(mm_sem, 1)
            # transposes: pooled [s, h] -> pooled_T [h, s], per tile
            tensor.wait_ge(csem["ident"], 16)
            for st in range(4):
                tensor.wait_ge(cp_sem, st + 1)
                for hb in range(2):
                    ins = tensor.transpose(
                        ps_t[hb][:, 128 * st : 128 * (st + 1)],
                        pool_sb[:, st, 128 * hb : 128 * (hb + 1)],
                        ident_sb[:, :],
                    )
                if st < 3:
                    ins.then_inc(tr_sem, 1)
                else:
                    tensor.matmul(
                        ps_x[:, 0:H], zlhs, zrhs, start=True, stop=True,
                        skip_group_check=True,
                    ).then_inc(tr_sem, 1)
            # Linear: out[s, j] = sum_h pooled_T[h, s] * wt[h, j]
            tensor.wait_ge(csem["wt"], 16)
            for st in range(4):
                tensor.wait_ge(cp2_sem, st + 1)
                tensor.matmul(
                    ps_s[st][:, 0:H],
                    sums2_sb[:, 0, st * 128 : (st + 1) * 128],
                    wt_sb[:, 0, :],
                    start=True,
                    stop=False,
                )
                tensor.matmul(
                    ps_s[st][:, 0:H],
                    sums2_sb[:, 1, st * 128 : (st + 1) * 128],
                    wt_sb[:, 1, :],
                    start=False,
                    stop=True,
                )
                tensor.matmul(
                    ps_x[:, 0:H], zlhs, zrhs, start=True, stop=True,
                    skip_group_check=True,
                ).then_inc(mme_sem, 1)

    return nc


def kernel(x, dst_idx, dst_size, W, b):
    x = np.asarray(x)
    idx = np.asarray(dst_idx).astype(np.int64)
    W = np.asarray(W, dtype=np.float32)
    b = np.asarray(b, dtype=np.float32)
    S = int(dst_size)
    assert S == S_TOTAL and x.shape[1] == H

    counts = np.bincount(idx, minlength=S).astype(np.float32)
    inv = np.float32(1.0) / (counts + EPS)  # [4096] f32

    order = np.argsort(idx, kind="stable")
    sidx = idx[order]
    bounds = np.searchsorted(sidx, np.arange(0, S + 1, S_PER))

    x16 = x.astype(np.float16)

    # split each core's rows into band A (rank < C), band B
    # (C <= rank < C+C2), and overflow (rank >= C+C2)
    bands, bands2, ovs, ovsegs = [], [], [], []
    for i in range(N_CORES):
        lo_i, hi_i = bounds[i], bounds[i + 1]
        n_i = hi_i - lo_i
        li = (sidx[lo_i:hi_i] - S_PER * i).astype(np.int64)
        rows = order[lo_i:hi_i]
        starts = np.searchsorted(li, np.arange(S_PER + 1))
        rank = np.arange(n_i) - starts[li]
        bm = rank < C
        sa = li[bm]
        slot = (16 * ((sa % 32) // 8) + sa // 32) * 128 + (sa % 8) * C + rank[bm]
        xband = np.zeros((128, KB, H), dtype=np.float16)
        xband[slot % 128, slot // 128] = x16[rows[bm]]
        bands.append(xband)
        bm2 = (rank >= C) & (rank < C + C2)
        sb = li[bm2]
        slot2 = (16 * ((sb % 32) // 16) + sb // 32) * 128 + (sb % 16) * C2 + (
            rank[bm2] - C
        )
        xband2 = np.zeros((128, KB2, H), dtype=np.float16)
        xband2[slot2 % 128, slot2 // 128] = x16[rows[bm2]]
        bands2.append(xband2)
        om = rank >= C + C2
        ovs.append(x16[rows[om]])
        ovsegs.append(li[om])

    ov_chunks = max(-(-len(s) // 128) for s in ovsegs)
    ovk = max(ov_chunks, 1)

    # shared overflow window schedule (32-aligned part starts)
    wins, parts = [], []
    for oc in range(ov_chunks):
        lo_w, hi_w = S_PER - 1, 0
        for s in ovsegs:
            seg = s[128 * oc : 128 * (oc + 1)]
            if len(seg):
                lo_w = min(lo_w, int(seg[0]))
                hi_w = max(hi_w, int(seg[-1]))
        hi_w = max(hi_w, lo_w)
        w = (lo_w // 32) * 32
        wins.append(w)
        parts.append(tuple(range(w, (hi_w // 32) * 32 + 32, 32)))
    wmax2 = max((len(p) for p in parts), default=1) * 32
    parts_t = tuple(parts)

    key = (ov_chunks, parts_t, wmax2)
    nc = _graph_cache.get(key)
    if nc is None:
        nc = _build(ov_chunks, parts_t, wmax2)
        _graph_cache[key] = nc

    iota_np = np.zeros((128, wmax2 + 256), dtype=np.float16)
    iota_np[:, :wmax2] = np.arange(wmax2, dtype=np.float16)
    ones_np = np.zeros((128, 6, 32), dtype=np.float16)
    r = np.arange(128)
    for v in range(4):
        ones_np[r, v, 8 * v + r // C] = 1.0
    for u in range(2):
        ones_np[r, 4 + u, 16 * u + r // C2] = 1.0
    ident_np = np.eye(128, dtype=np.float16)
    wt_np = np.ascontiguousarray(W.T).astype(np.float16)
    bb_np = np.ascontiguousarray(np.tile(b, (128, 1)), dtype=np.float32)

    in_maps = []
    for i in range(N_CORES):
        n_ov = len(ovsegs[i])
        xov = np.zeros((128, ovk, H), dtype=np.float16)
        ro = np.arange(n_ov)
        xov[ro % 128, ro // 128] = ovs[i]
        ovidx = np.full((128, ovk), PAD_IDX, dtype=np.float32)
        if ov_chunks:
            ovidx[ro % 128, ro // 128] = ovsegs[i] - np.repeat(wins, 128)[:n_ov]
        invc_np = np.ascontiguousarray(
            inv[S_PER * i : S_PER * (i + 1)].reshape(4, 128).T
        )
        in_maps.append(
            {
                "xb": bands[i],
                "xb2": bands2[i],
                "xov": xov,
                "ovidx": ovidx,
                "iota": iota_np,
                "ones32": ones_np,
                "ident": ident_np,
                "wt": wt_np,
                "invc": invc_np,
                "bb": bb_np,
            }
        )

    res = run_bass_kernel_spmd(nc, in_maps, core_ids=list(range(N_CORES)))
    return np.concatenate([res.results[i]["out"] for i in range(N_CORES)], axis=0)



# revision 4
# speedup vs baseline: 1.1379x; 1.1379x over previous
"""Segment-mean pooling (segment_sum / counts) + Linear, on 8 TRN2 NeuronCores.

Strategy: segment-ownership sharding.  The host sorts rows by dst_idx and
routes each row to the core that owns its segment range (core i owns
segments [512*i, 512*(i+1))), so no collectives are needed; the host
concatenates the 8 output shards.

Per core the 512 segments split into 4 tiles of 128 segments.  All
accumulation matmuls are full-width M=128 (stationary is a [128, 128]
one-hot), which keeps the PE HAM activity monitor fed so the clock stays
at 2.4 GHz, and every 128-row chunk of x costs exactly one N=256 matmul:

  Band pass: the host packs the first C=16 rows of every segment into a
  dense band (fill ~99%); chunk c covers segs [8c, 8c+8) and its
  stationary is one of 16 fixed patterns (built on-device by DVE
  is_equal against an iota row).

  Tail pass: rows with rank >= 16 are packed densely in segment order,
  split at 128-segment tile boundaries so each tail chunk maps into one
  PSUM tile.  Each chunk ships a [128] relative-segment-index vector;
  DVE builds its [128, 128] one-hot, and one matmul accumulates it.
  Chunk counts per tile are maxed across cores (SPMD graph identity);
  short cores pad with zero rows / relidx=999 (one-hot of zeros).

Throughput notes (from baseline trace analysis): each dma_start costs
~650ns of issue time on its HWDGE ring, so data ships as ~11 large
transfers; semaphore count is minimized (8) because program teardown
costs ~2 ops per semaphore per engine; the PE is warmed with ~3.4us of
junk matmuls at t=0 so the band pass runs at 2.4 GHz, not 1.2.

Epilogue per tile (pipelined under the DMA stream of later tiles):
fence, DVE-copy pooled sums to SBUF f16, PE-transpose to [h, s], apply
the Linear as out[s, j] = sums_T[:, s].T @ W.T[h, j], then one fused DVE
op scales rows by 1/(count+eps) (host bincount reciprocal) and adds the
bias, and sync-ring DMAs the [128, 256] f32 tile out.  Only the last
tile's epilogue is exposed past the end of the input stream.
"""

import numpy as np

import concourse.bass as bass
import concourse.mybir as mybir
from concourse.bass_utils import run_bass_kernel_spmd

N_CORES = 8
S_TOTAL = 4096
S_PER = S_TOTAL // N_CORES  # 512 segments per core
H = 256
EPS = np.float32(1e-8)
C = 16  # band capacity (rows per segment); must divide 128
KB = S_PER * C // 128  # 64 band chunks
NTILE = 4  # 128-seg tiles per core
PAD_IDX = 999.0  # relidx sentinel; never matches iota [0, 128)

_graph_cache: dict = {}


def _build(NT: tuple) -> "bass.Bass":
    """NT[t] = tail chunks for tile t (shared across cores)."""
    f16 = mybir.dt.float16
    f32 = mybir.dt.float32
    NTsum = sum(NT)
    ncol_f = 20 + H + NTsum  # patsc16 | invc4 | bb256 | relidx

    nc = bass.Bass()

    xb_d = nc.declare_dram_parameter("xb", [128, KB, H], f16, isOutput=False)
    xt_d = nc.declare_dram_parameter("xt", [128, NTsum, H], f16, isOutput=False)
    ch_d = nc.declare_dram_parameter("ch", [128, 768], f16, isOutput=False)
    cf_d = nc.declare_dram_parameter("cf", [128, ncol_f], f32, isOutput=False)
    out_d = nc.declare_dram_parameter("out", [S_PER, H], f32, isOutput=True)

    from contextlib import ExitStack

    with ExitStack() as ctx:
        xbb = ctx.enter_context(nc.sbuf_tensor("xbb", [128, KB, H], f16))
        xtt = ctx.enter_context(nc.sbuf_tensor("xtt", [128, NTsum, H], f16))
        ch = ctx.enter_context(nc.sbuf_tensor("ch_sb", [128, 768], f16))
        cf = ctx.enter_context(nc.sbuf_tensor("cf_sb", [128, ncol_f], f32))
        pat = ctx.enter_context(nc.sbuf_tensor("pat", [128, 16, 128], f16))
        oh = ctx.enter_context(nc.sbuf_tensor("oh", [128, NTsum, 128], f16))
        pool = ctx.enter_context(nc.sbuf_tensor("pool", [128, NTILE, H], f16))
        sums2 = ctx.enter_context(nc.sbuf_tensor("sums2", [128, 2, 128], f16))
        outb = ctx.enter_context(nc.sbuf_tensor("outb", [128, NTILE, H], f32))
        # every PSUM tensor is one full private 2 KiB bank
        ps_s = [
            ctx.enter_context(nc.psum_tensor(f"ps_s{t}", [128, 512], f32))
            for t in range(NTILE)
        ]
        ps_t = [
            ctx.enter_context(nc.psum_tensor(f"ps_t{hb}", [128, 1024], f16))
            for hb in range(2)
        ]
        ps_x = ctx.enter_context(nc.psum_tensor("ps_x", [128, 512], f32))
        sconst = ctx.enter_context(nc.semaphore("sconst"))
        sdat = [ctx.enter_context(nc.semaphore(f"sdat{g}")) for g in range(4)]
        s_pe = ctx.enter_context(nc.semaphore("s_pe"))
        s_dv = ctx.enter_context(nc.semaphore("s_dv"))
        s_od = ctx.enter_context(nc.semaphore("s_od"))
        block = ctx.enter_context(nc.Block())

        zlhs = ch[0:1, 0:128]  # junk 1-partition stationary for fences
        zrhs = ch[0:1, 256:512]  # junk rhs; ps_x is never read

        # s_dv value map (DVE producer)
        d_pat = 16
        d_oh = [16 + sum(NT[:t + 1]) for t in range(NTILE)]
        d_cp = [16 + NTsum + 3 * t + 1 for t in range(NTILE)]
        d_cp2 = [16 + NTsum + 3 * t + 2 for t in range(NTILE)]
        d_oe = [16 + NTsum + 3 * t + 3 for t in range(NTILE)]
        # s_pe value map (PE producer): fence_t, tr_t, lin_t
        p_fence = [1, 3, 6, 9]
        p_tr = [2, 5, 8, 11]
        p_lin = [4, 7, 10, 12]
        # data DMA -> (sem idx, threshold), issue order:
        # band0 tail0 band1 tail1 band2 tail2 band3 tail3a tail3b
        g_band = [(0, 16), (2, 16), (0, 32), (2, 32)]
        g_tail = [(1, 16), (3, 16), (1, 32), (3, 32)]
        g_t3b = (0, 48)
        nt3a = NT[3] - 2 if NT[3] >= 3 else NT[3]
        toff = [sum(NT[:t]) for t in range(NTILE)]

        @block.scalar
        def _(scalar):
            # all input DMAs, one ring (q10), in consumption order
            scalar.dma_start(out=ch[:, :], in_=ch_d[:, :]).then_inc(sconst, 16)
            scalar.dma_start(out=cf[:, :], in_=cf_d[:, :]).then_inc(sconst, 16)
            for t in range(NTILE):
                scalar.dma_start(
                    out=xbb[:, 16 * t : 16 * (t + 1), :],
                    in_=xb_d[:, 16 * t : 16 * (t + 1), :],
                ).then_inc(sdat[g_band[t][0]], 16)
                if t < 3 or nt3a == NT[3]:
                    scalar.dma_start(
                        out=xtt[:, toff[t] : toff[t] + NT[t], :],
                        in_=xt_d[:, toff[t] : toff[t] + NT[t], :],
                    ).then_inc(sdat[g_tail[t][0]], 16)
                else:
                    scalar.dma_start(
                        out=xtt[:, toff[3] : toff[3] + nt3a, :],
                        in_=xt_d[:, toff[3] : toff[3] + nt3a, :],
                    ).then_inc(sdat[g_tail[3][0]], 16)
                    scalar.dma_start(
                        out=xtt[:, toff[3] + nt3a : toff[3] + NT[3], :],
                        in_=xt_d[:, toff[3] + nt3a : toff[3] + NT[3], :],
                    ).then_inc(sdat[g_t3b[0]], 16)

        @block.vector
        def _(vector):
            vector.wait_ge(sconst, 32)
            for c in range(16):
                vector.tensor_scalar(
                    out=pat[:, c, :],
                    in0=ch[:, 0:128],
                    scalar1=cf[:, c : c + 1],
                    scalar2=None,
                    op0=mybir.AluOpType.is_equal,
                ).then_inc(s_dv, 1)
            for k in range(NTsum):
                vector.tensor_scalar(
                    out=oh[:, k, :],
                    in0=ch[:, 0:128],
                    scalar1=cf[:, 20 + H + k : 21 + H + k],
                    scalar2=None,
                    op0=mybir.AluOpType.is_equal,
                ).then_inc(s_dv, 1)
            for t in range(NTILE):
                vector.wait_ge(s_pe, p_fence[t])
                vector.tensor_copy(out=pool[:, t, :], in_=ps_s[t][:, 0:H]).then_inc(
                    s_dv, 1
                )
                vector.wait_ge(s_pe, p_tr[t])
                vector.tensor_copy(out=sums2[:, 0, :], in_=ps_t[0][:, 0:128])
                vector.tensor_copy(
                    out=sums2[:, 1, :], in_=ps_t[1][:, 0:128]
                ).then_inc(s_dv, 1)
                vector.wait_ge(s_pe, p_lin[t])
                vector.scalar_tensor_tensor(
                    out=outb[:, t, :],
                    in0=ps_s[t][:, 0:H],
                    scalar=cf[:, 16 + t : 17 + t],
                    in1=cf[:, 20 : 20 + H],
                    op0=mybir.AluOpType.mult,
                    op1=mybir.AluOpType.add,
                ).then_inc(s_dv, 1)

        @block.tensor
        def _(tensor):
            def fence(inc=True):
                ins = tensor.matmul(
                    ps_x[:, 0:H], zlhs, zrhs, start=True, stop=True,
                    skip_group_check=True,
                )
                if inc:
                    ins.then_inc(s_pe, 1)

            def band(t):
                tensor.wait_ge(sdat[g_band[t][0]], g_band[t][1])
                for i in range(16):
                    tensor.matmul(
                        ps_s[t][:, 0:H],
                        pat[:, i, :],
                        xbb[:, 16 * t + i, :],
                        start=(i == 0),
                        stop=(i == 15 and NT[t] == 0),
                        skip_group_check=True,
                    )

            def tail(t):
                if t < 3 or nt3a == NT[3]:
                    tensor.wait_ge(sdat[g_tail[t][0]], g_tail[t][1])
                    parts = [range(NT[t])]
                else:
                    parts = [range(nt3a), range(nt3a, NT[3])]
                    tensor.wait_ge(sdat[g_tail[3][0]], g_tail[3][1])
                tensor.wait_ge(s_dv, d_oh[t])
                for pi, rng in enumerate(parts):
                    if pi == 1:
                        tensor.wait_ge(sdat[g_t3b[0]], g_t3b[1])
                    for k in rng:
                        tensor.matmul(
                            ps_s[t][:, 0:H],
                            oh[:, toff[t] + k, :],
                            xtt[:, toff[t] + k, :],
                            start=False,
                            stop=(k == NT[t] - 1),
                            skip_group_check=True,
                        )

            def trans(t):
                tensor.wait_ge(s_dv, d_cp[t])
                for hb in range(2):
                    tensor.transpose(
                        ps_t[hb][:, 0:128],
                        pool[:, t, 128 * hb : 128 * (hb + 1)],
                        ch[:, 128:256],
                    )
                fence()  # drain guard before DVE reads ps_t -> inc s_pe

            def linear(t):
                tensor.wait_ge(s_dv, d_cp2[t])
                tensor.matmul(
                    ps_s[t][:, 0:H], sums2[:, 0, :], ch[:, 256:512],
                    start=True, stop=False, skip_group_check=True,
                )
                tensor.matmul(
                    ps_s[t][:, 0:H], sums2[:, 1, :], ch[:, 512:768],
                    start=False, stop=True, skip_group_check=True,
                )
                fence()

            # ~3.4us of sustained full-width matmuls: HAM -> 2.4 GHz
            tensor.wait_ge(sconst, 16)
            for _ in range(8):
                tensor.matmul(
                    ps_x[:, 0:512], ch[:, 0:128], ch[:, 0:512],
                    start=True, stop=True, skip_group_check=True,
                )
            tensor.wait_ge(s_dv, d_pat)
            band(0)
            tail(0)
            fence()
            band(1)
            trans(0)
            tail(1)
            fence()
            linear(0)
            band(2)
            trans(1)
            tail(2)
            fence()
            linear(1)
            band(3)
            trans(2)
            tail(3)
            fence()
            linear(2)
            trans(3)
            linear(3)

        @block.sync
        def _(sync):
            for t in range(NTILE):
                sync.wait_ge(s_dv, d_oe[t])
                sync.dma_start(
                    out=out_d[128 * t : 128 * (t + 1), :], in_=outb[:, t, :]
                ).then_inc(s_od, 16)
            sync.wait_ge(s_od, 64)

    return nc


def kernel(x, dst_idx, dst_size, W, b):
    x = np.asarray(x)
    idx = np.asarray(dst_idx).astype(np.int64)
    W = np.asarray(W, dtype=np.float32)
    b = np.asarray(b, dtype=np.float32)
    S = int(dst_size)
    assert S == S_TOTAL and x.shape[1] == H

    counts = np.bincount(idx, minlength=S).astype(np.float32)
    inv = np.float32(1.0) / (counts + EPS)  # [4096] f32

    order = np.argsort(idx, kind="stable")
    sidx = idx[order]
    bounds = np.searchsorted(sidx, np.arange(0, S + 1, S_PER))

    x16 = x.astype(np.float16)

    # split each core's rows into band (rank < C) and tail (rank >= C)
    bands, tails, tsegs = [], [], []
    for i in range(N_CORES):
        lo_i, hi_i = bounds[i], bounds[i + 1]
        n_i = hi_i - lo_i
        li = (sidx[lo_i:hi_i] - S_PER * i).astype(np.int64)
        rows = order[lo_i:hi_i]
        starts = np.searchsorted(li, np.arange(S_PER + 1))
        rank = np.arange(n_i) - starts[li]
        bm = rank < C
        sa = li[bm]
        xband = np.zeros((128, KB, H), dtype=np.float16)
        xband[(sa % 8) * 16 + rank[bm], sa // 8] = x16[rows[bm]]
        bands.append(xband)
        tm = ~bm
        tails.append(x16[rows[tm]])
        tsegs.append(li[tm])

    # shared per-tile tail chunk counts (graph identity across cores)
    NT = []
    for t in range(NTILE):
        m = max(
            int(((s >= 128 * t) & (s < 128 * (t + 1))).sum()) for s in tsegs
        )
        NT.append(-(-m // 128))
    NT = tuple(NT)
    NTsum = sum(NT)
    toff = [sum(NT[:t]) for t in range(NTILE)]

    key = NT
    nc = _graph_cache.get(key)
    if nc is None:
        nc = _build(NT)
        _graph_cache[key] = nc

    # shared f16 consts: iota | ident | W.T packed per h-half
    ch_np = np.zeros((128, 768), dtype=np.float16)
    ch_np[:, 0:128] = np.arange(128, dtype=np.float16)
    ch_np[:, 128:256] = np.eye(128, dtype=np.float16)
    WT = np.ascontiguousarray(W.T).astype(np.float16)  # [h, j]
    ch_np[:, 256:512] = WT[0:128, :]
    ch_np[:, 512:768] = WT[128:256, :]
    patsc = (8 * np.arange(16)[None, :] + (np.arange(128) // C)[:, None]).astype(
        np.float32
    )
    bbt = np.tile(b, (128, 1)).astype(np.float32)

    in_maps = []
    for i in range(N_CORES):
        xt_np = np.zeros((128, NTsum, H), dtype=np.float16)
        relidx = np.full((128, NTsum), PAD_IDX, dtype=np.float32)
        s_i, x_i = tsegs[i], tails[i]
        for t in range(NTILE):
            m = (s_i >= 128 * t) & (s_i < 128 * (t + 1))
            st, xt_rows = s_i[m], x_i[m]
            r = np.arange(len(st))
            xt_np[r % 128, toff[t] + r // 128] = xt_rows
            relidx[r % 128, toff[t] + r // 128] = st - 128 * t
        cf_np = np.zeros((128, 20 + H + NTsum), dtype=np.float32)
        cf_np[:, 0:16] = patsc
        cf_np[:, 16:20] = inv[S_PER * i : S_PER * (i + 1)].reshape(4, 128).T
        cf_np[:, 20 : 20 + H] = bbt
        cf_np[:, 20 + H :] = relidx
        in_maps.append(
            {"xb": bands[i], "xt": xt_np, "ch": ch_np, "cf": cf_np}
        )

    res = run_bass_kernel_spmd(nc, in_maps, core_ids=list(range(N_CORES)))
    return np.concatenate([res.results[i]["out"] for i in range(N_CORES)], axis=0)
